# revision 1
# baseline (speedup 1.0000x reference)
"""CPC loss kernel for Trainium2 (Bass/Tile), data-parallel over batch on 8 NeuronCores.

Math: the reference computes, per forward step i = k+1 (k = 0..K-1):
    step_loss_k = -mean_{b, t in [0, T-i)} log(pos / neg)
with pos = exp(sum_e ce*be), neg = exp(sum_n sum_e ce*neg_n), so
    log(pos/neg) = sum_e ce[b,t,e] * (base[b,t+i,e] - negsum[b,e])
where ce = mask[b,t] * mapped_ctx[b,t,e,k] and negsum[b] = sum_n neg_samples[b,n].
The exp/log cancel exactly; the whole loss is a masked sum of dot products.

Device layout: e (=128) on partitions, t on the free dim, so the per-step shift
t -> t+k+1 is a free-dim offset. Per batch row, one 2-4MB DMA brings all K
mapped_ctx planes in. Per (row, k): one DVE multiply mctxT[e, t]*bmnT[e, t+k+1],
then a TensorE matmul whose stationary is a one-hot column (slice of a
precomputed "wide" matrix) reduces over partitions (e), landing the column sums
in PSUM partition r*K+k (zeros accumulate in the other partitions). A final
tensor_mul + reduce_sum applies the seq-len mask weights and reduces over t.
Host applies the per-step 1/(B*(T-i)) scaling.

Modes: "f32x" = all fp32 (exact, fp32 matmul at 1/4 rate);
       "f32"  = fp32 inputs, products rounded to float32r (full PE rate);
       "bf16" = bf16 inputs/products (halves DMA, 2x DVE).
"""

import numpy as np

B, T, E, K, NNEG = 64, 1024, 128, 8, 64
NCORES = 8
B_LOC = B // NCORES          # batch rows per core
TPAD = T + 8                 # bmn padded along t so every shifted read is in-bounds
L = T                        # compute width; t=T-1 column is always masked

MODE = "fp16"                # "f32x" | "f32" | "bf16" | "fp16"
_CACHE = {}
TRACE = False                # test harness may flip this for NTFF profiling
TRACE_KWARGS = {}
LAST_RESULTS = None


def _build(mode):
    from contextlib import ExitStack
    import concourse.bass as bass
    import concourse.bacc as bacc
    import concourse.tile as tile
    import concourse.mybir as mybir

    f32 = mybir.dt.float32
    cdt = {"f32x": f32, "f32": mybir.dt.float32r, "bf16": mybir.dt.bfloat16,
           "fp16": mybir.dt.float16}[mode]
    in_dt = cdt if mode in ("bf16", "fp16") else f32

    nc = bacc.Bacc(
        "TRN2",
        target_bir_lowering=False,
        debug=False,
        enable_asserts=False,
        num_devices=NCORES,
    )
    mctx_in = nc.dram_tensor("mctxT", [B_LOC, E, K, T], in_dt, kind="ExternalInput").ap()
    bmn_in = nc.dram_tensor("bmnT", [B_LOC, E, TPAD], in_dt, kind="ExternalInput").ap()
    w_in = nc.dram_tensor("w", [B_LOC * K, L], f32, kind="ExternalInput").ap()
    wide_in = nc.dram_tensor("wide", [E, 127], cdt, kind="ExternalInput").ap()
    s_out = nc.dram_tensor("S", [B_LOC * K, 1], f32, kind="ExternalOutput").ap()

    with tile.TileContext(nc) as tc, ExitStack() as ctx:
        m_pool = ctx.enter_context(tc.tile_pool(name="m", bufs=5))
        bmn_pool = ctx.enter_context(tc.tile_pool(name="bmn", bufs=3))
        prod_pool = ctx.enter_context(tc.tile_pool(name="prod", bufs=3))
        misc_pool = ctx.enter_context(tc.tile_pool(name="misc", bufs=1))
        psum_pool = ctx.enter_context(tc.tile_pool(name="psum", bufs=1, space="PSUM"))

        NR = B_LOC * K  # 64 psum rows, one per (r, k)
        # wide[:, 63] = 1, else 0. Slicing wide[:, 63-row : 127-row] gives a
        # [128, 64] one-hot-column stationary that lands the column sum of the
        # moving operand in PSUM partition `row` (zeros accumulate elsewhere).
        wide = misc_pool.tile([E, 127], cdt)
        nc.scalar.dma_start(wide[:], wide_in[:, :])
        wt = misc_pool.tile([NR, L], f32)
        nc.gpsimd.dma_start(wt[:], w_in[:, :])
        ps = psum_pool.tile([NR, L], f32)

        KH = K // 2
        for r in range(B_LOC):
            # two half-row DMAs on separate HWDGE queues (sync + scalar) so
            # they stream concurrently and the first muls start after ~1MB
            bmn = bmn_pool.tile([E, TPAD], in_dt)
            nc.sync.dma_start(bmn[:], bmn_in[r])
            m_lo = m_pool.tile([E, KH, T], in_dt, tag="m_lo")
            nc.sync.dma_start(m_lo[:], mctx_in[r, :, 0:KH, :])
            m_hi = m_pool.tile([E, KH, T], in_dt, tag="m_hi")
            nc.scalar.dma_start(m_hi[:], mctx_in[r, :, KH:K, :])
            if mode in ("bf16", "fp16"):
                # bmn_s[e, t] = bmn[e, t+1]: gives 4B-aligned window bases for
                # the even-k (odd-shift) fused multiply below.
                bmn_s = bmn_pool.tile([E, TPAD], in_dt, tag="bmn_s")
                nc.gpsimd.dma_start(bmn_s[:, 0:TPAD - 1], bmn[:, 1:TPAD])
            # Two fused multiplies per row, one per k-parity group (host lays
            # out planes in korder = [1,3,5,7,0,2,4,6]). The bmn operand is a
            # 3D AP of 4 overlapping shifted windows (k-dim step 2 elements).
            for half, m_half in (("lo", m_lo), ("hi", m_hi)):
                prod = prod_pool.tile([E, KH, T], cdt, tag=f"prod_{half}")
                if half == "lo":
                    # korig 1,3,5,7 -> shifts 2,4,6,8 (even, aligned)
                    src = bass.AP(bmn[:].tensor, 2, [[TPAD, E], [2, KH], [1, T]])
                elif mode in ("bf16", "fp16"):
                    # korig 0,2,4,6 -> shifts 1,3,5,7 via bmn_s at 0,2,4,6
                    src = bass.AP(bmn_s[:].tensor, 0, [[TPAD, E], [2, KH], [1, T]])
                else:
                    src = bass.AP(bmn[:].tensor, 1, [[TPAD, E], [2, KH], [1, T]])
                nc.vector.tensor_mul(prod[:, :, :], m_half[:, :, :], src)
                for j in range(KH):
                    row = r * K + (j if half == "lo" else KH + j)
                    oh = wide[:, NR - 1 - row:2 * NR - 1 - row]
                    first = row == 0
                    last = row == NR - 1
                    nc.tensor.matmul(
                        ps[:, 0:512], lhsT=oh,
                        rhs=prod[:, j, 0:512], start=first, stop=last,
                    )
                    nc.tensor.matmul(
                        ps[:, 512:L], lhsT=oh,
                        rhs=prod[:, j, 512:L], start=first, stop=last,
                    )

            if r == B_LOC // 2 - 1:
                # first-half finisher overlaps the remaining rows' compute
                scratch = misc_pool.tile([NR, L], f32)
                s_tile = misc_pool.tile([NR, 1], f32)
                half_rows = (B_LOC // 2) * K
                nc.vector.tensor_mul(
                    scratch[0:half_rows, :], ps[0:half_rows, :], wt[0:half_rows, :])
                nc.vector.reduce_sum(
                    s_tile[0:half_rows, :], scratch[0:half_rows, :],
                    axis=mybir.AxisListType.X)
                nc.scalar.dma_start(s_out[0:half_rows, :], s_tile[0:half_rows, :])

        nc.vector.tensor_mul(
            scratch[half_rows:NR, :], ps[half_rows:NR, :], wt[half_rows:NR, :])
        nc.vector.reduce_sum(
            s_tile[half_rows:NR, :], scratch[half_rows:NR, :],
            axis=mybir.AxisListType.X)
        nc.scalar.dma_start(s_out[half_rows:NR, :], s_tile[half_rows:NR, :])

    nc.compile()
    return nc


def kernel(base_emb, mapped_ctx, seq_lens, neg_ids):
    global LAST_RESULTS
    import ml_dtypes
    from concourse import bass_utils

    base = np.ascontiguousarray(np.asarray(base_emb, dtype=np.float32))
    mctx = np.asarray(mapped_ctx, dtype=np.float32)
    seq = np.asarray(seq_lens, dtype=np.int32)
    nids = np.asarray(neg_ids, dtype=np.int32)

    np_in_dt = {"bf16": ml_dtypes.bfloat16, "fp16": np.float16}.get(MODE, np.float32)

    # Host prep (sharding + per-batch-element negative gather, per sharding hint)
    neg_sum = base.reshape(B * T, E)[nids].sum(axis=1)             # [B, E]
    bmn = base - neg_sum[:, None, :]                               # [B, T, E]
    bmnT = np.zeros((B, E, TPAD), np_in_dt)
    bmnT[:, :, :T] = bmn.transpose(0, 2, 1)
    korder = [1, 3, 5, 7, 0, 2, 4, 6]
    mctxT = np.ascontiguousarray(
        mctx.transpose(0, 2, 3, 1)[:, :, korder, :].astype(np_in_dt))

    t_idx = np.arange(L)[None, None, :]                            # [1, 1, L]
    lim = np.minimum(seq[:, None], (T - 1 - np.arange(K))[None, :])  # [B, K]
    w = (t_idx < lim[:, :, None]).astype(np.float32)[:, korder, :]  # [B, K, L]
    wide = np.zeros((E, 127), np_in_dt if MODE in ("bf16", "fp16") else np.float32)
    wide[:, 63] = 1.0

    key = ("nc", MODE)
    if key not in _CACHE:
        _CACHE[key] = _build(MODE)
    nc = _CACHE[key]

    in_maps = []
    for c in range(NCORES):
        sl = slice(c * B_LOC, (c + 1) * B_LOC)
        in_maps.append({
            "mctxT": mctxT[sl],
            "bmnT": np.ascontiguousarray(bmnT[sl]),
            "w": np.ascontiguousarray(w[sl].reshape(B_LOC * K, L)),
            "wide": wide,
        })

    res = bass_utils.run_bass_kernel_spmd(
        nc, in_maps, core_ids=list(range(NCORES)), trace=TRACE, **TRACE_KWARGS
    )
    LAST_RESULTS = res

    S_dev = np.concatenate([r["S"].reshape(B_LOC, K) for r in res.results])  # [B, K(korder)]
    loss = 0.0
    for j, korig in enumerate(korder):
        loss += -S_dev[:, j].sum(dtype=np.float64) / (B * (T - korig - 1))
    loss /= K
    return np.float32(loss)



# revision 3
# speedup vs baseline: 1.3283x; 1.3283x over previous
"""CPC loss kernel for Trainium2 (Bass/Tile), data-parallel over batch on 8 NeuronCores.

Math: the reference's exp/log cancel exactly; the loss is a masked sum of dot
products: step_loss_k = -1/(B(T-i)) * sum_{b,t<lim} sum_e mctx[b,t,e,k]*bmn[b,t+i,e]
with i = k+1, lim = min(seq_len[b], T-i), bmn = base - sum_n neg_samples.

Device layout: e (=128) on partitions, t on the free dim. Per (row, k): DVE
multiplies mctxT[e, t]*bmn[e, t+k+1] (the shift is a free-dim AP offset), then a
TensorE matmul with a one-hot-column stationary reduces over partitions, landing
column sums in PSUM partition row r*K+k (zeros accumulate elsewhere). A final
reduce_sum over t gives per-(row,k) scalars; host applies the -1/(B(T-i)) scales.

v2 over baseline:
 - Rows sorted by seq_len desc, assigned (slot s, core c) = rank s*8+c, so all
   cores share slot widths Ls = seq-derived (JIT-specialized program). DMA and
   compute are sliced to Ls (~75% of T on average).
 - Host zeroes mctx tails (t >= lim) so no mask tensor / mask-multiply needed;
   PSUM columns beyond a slot's Ls stay zero via the one-hot scheme (slot 0 is
   widest and initializes the full PSUM width with start=True).
 - The shift-by-1 copy of bmn (for odd-shift 4B alignment) runs on ScalarE
   instead of an SBUF->SBUF DMA, removing ~2.1MB of SDMA traffic.
 - DMA queues balanced: m_lo on sync, m_hi on scalar, bmn on gpsimd.
 - Slot 0 is panel-split (2x512 cols per half) so compute starts earlier.
"""

import numpy as np

B, T, E, K, NNEG = 64, 1024, 128, 8, 64
NCORES = 8
NSLOT = B // NCORES          # 8 slots (one row per slot per core)
KH = K // 2
KORDER = [1, 3, 5, 7, 0, 2, 4, 6]   # lo half: shifts 2,4,6,8 ; hi half: 1,3,5,7
PAD = 16                     # bmn width pad so every shifted window is in-bounds

MODE = "fp16"
_CACHE = {}
TRACE = False
TRACE_KWARGS = {}
LAST_RESULTS = None


def _build(slot_lens):
    from contextlib import ExitStack
    import concourse.bass as bass
    import concourse.bacc as bacc
    import concourse.tile as tile
    import concourse.mybir as mybir

    f32 = mybir.dt.float32
    f16 = mybir.dt.float16
    L0 = slot_lens[0]
    NR = NSLOT * K

    nc = bacc.Bacc(
        "TRN2",
        target_bir_lowering=False,
        debug=False,
        enable_asserts=False,
        num_devices=NCORES,
    )
    m_in = []
    bmn_in = []
    for s, Ls in enumerate(slot_lens):
        m_in.append(nc.dram_tensor(f"m{s}", [E, K, Ls], f16, kind="ExternalInput").ap())
        bmn_in.append(
            nc.dram_tensor(f"bmn{s}", [E, Ls + PAD], f16, kind="ExternalInput").ap())
    wide_in = nc.dram_tensor("wide", [E, 2 * NR - 1], f16, kind="ExternalInput").ap()
    s_out = nc.dram_tensor("S", [NR, 1], f32, kind="ExternalOutput").ap()

    with tile.TileContext(nc) as tc, ExitStack() as ctx:
        m_pool = ctx.enter_context(tc.tile_pool(name="m", bufs=3))
        bmn_pool = ctx.enter_context(tc.tile_pool(name="bmn", bufs=2))
        prod_pool = ctx.enter_context(tc.tile_pool(name="prod", bufs=2))
        misc_pool = ctx.enter_context(tc.tile_pool(name="misc", bufs=1))
        psum_pool = ctx.enter_context(tc.tile_pool(name="psum", bufs=1, space="PSUM"))

        # wide[:, NR-1] = 1, else 0. wide[:, NR-1-row : 2*NR-1-row] is a
        # ones-column at position `row`: the matmul lands the column sum of the
        # moving operand in PSUM partition `row`, zeros elsewhere.
        wide = misc_pool.tile([E, 2 * NR - 1], f16)
        nc.gpsimd.dma_start(wide[:], wide_in[:, :])
        ps = psum_pool.tile([NR, L0], f32)

        def panels_of(s, Ls):
            if s == 0:
                return [(0, 512)] + ([(512, Ls)] if Ls > 512 else [])
            return [(0, Ls)]

        WBMAX = L0 + PAD
        for s, Ls in enumerate(slot_lens):
            WB = Ls + PAD
            bmn = bmn_pool.tile([E, WBMAX], f16, tag="bmn")
            nc.gpsimd.dma_start(bmn[:, 0:WB], bmn_in[s])
            # bmn_s[e, t] = bmn[e, t+1]: 4B-aligned bases for odd shifts.
            bmn_s = bmn_pool.tile([E, WBMAX], f16, tag="bmn_s")
            nc.scalar.copy(bmn_s[:, 0:WB - 1], bmn[:, 1:WB])

            panels = panels_of(s, Ls)
            for half, eng in (("lo", nc.sync), ("hi", nc.scalar)):
                khi = KH if half == "lo" else K
                for pi, (c0, c1) in enumerate(panels):
                    tsuf = f"{half}_{pi}_s0" if s == 0 else f"{half}_full"
                    m_t = m_pool.tile([E, KH, 512 if s == 0 else L0], f16,
                                      tag=f"m_{tsuf}")
                    eng.dma_start(
                        m_t[:, :, 0:c1 - c0],
                        m_in[s][:, khi - KH:khi, c0:c1])
                    prod = prod_pool.tile([E, KH, 512 if s == 0 else L0], f16,
                                          tag=f"prod_{tsuf}")
                    if half == "lo":
                        # shifts 2,4,6,8: bmn window base 2+c0, k-step 2
                        src = bass.AP(bmn[:].tensor, 2 + c0,
                                      [[WBMAX, E], [2, KH], [1, c1 - c0]])
                    else:
                        # shifts 1,3,5,7 via bmn_s at 0,2,4,6
                        src = bass.AP(bmn_s[:].tensor, c0,
                                      [[WBMAX, E], [2, KH], [1, c1 - c0]])
                    nc.vector.tensor_mul(
                        prod[:, :, 0:c1 - c0], m_t[:, :, 0:c1 - c0], src)
                    for j in range(KH):
                        row = s * K + (j if half == "lo" else KH + j)
                        oh = wide[:, NR - 1 - row:2 * NR - 1 - row]
                        for (d0, d1) in ([(c0, c1)] if s == 0
                                         else [(0, 512), (512, Ls)]):
                            if d1 <= d0:
                                continue
                            first = s == 0 and row == 0
                            last = (s == NSLOT - 1 and row == NR - 1
                                    and d1 == Ls)
                            nc.tensor.matmul(
                                ps[:, d0:d1], lhsT=oh,
                                rhs=prod[:, j, d0 - c0:d1 - c0],
                                start=first, stop=last,
                                skip_group_check=True,
                            )

        s_tile = misc_pool.tile([NR, 1], f32)
        nc.vector.reduce_sum(s_tile[:, :], ps[:, 0:L0], axis=mybir.AxisListType.X)
        nc.gpsimd.dma_start(s_out[:, :], s_tile[:, :])

    nc.compile()
    return nc


def kernel(base_emb, mapped_ctx, seq_lens, neg_ids):
    global LAST_RESULTS
    from concourse import bass_utils

    base = np.ascontiguousarray(np.asarray(base_emb, dtype=np.float32))
    mctx = np.asarray(mapped_ctx, dtype=np.float32)
    seq = np.asarray(seq_lens, dtype=np.int32)
    nids = np.asarray(neg_ids, dtype=np.int32)

    # Host prep: per-batch negative gather (per sharding hint), bmn = base - negsum
    neg_sum = base.reshape(B * T, E)[nids].sum(axis=1)             # [B, E]
    bmn = (base - neg_sum[:, None, :]).astype(np.float16)          # [B, T, E]

    # Row -> (slot, core) assignment: sort by needed width desc; slot s takes
    # ranks [8s, 8s+8), one per core. All cores share slot widths.
    lim = np.minimum(seq[:, None], (T - 1 - np.arange(K))[None, :])  # [B, K] per korig
    need = lim.max(axis=1)                                           # [B]
    order = np.argsort(-need, kind="stable")                         # rank -> b
    slot_lens = []
    for s in range(NSLOT):
        group = order[s * NCORES:(s + 1) * NCORES]
        Ls = int(need[group].max())
        Ls = min(T, max(512, -(-Ls // 64) * 64))
        if s == 0:
            Ls = min(T, -(-Ls // 512) * 512)
        slot_lens.append(Ls)
    slot_lens = tuple(slot_lens)

    wide = np.zeros((E, 2 * NSLOT * K - 1), np.float16)
    wide[:, NSLOT * K - 1] = 1.0

    key = ("nc", MODE, slot_lens)
    if key not in _CACHE:
        _CACHE[key] = _build(slot_lens)
    nc = _CACHE[key]

    in_maps = [{"wide": wide} for _ in range(NCORES)]
    for s in range(NSLOT):
        Ls = slot_lens[s]
        for c in range(NCORES):
            b = int(order[s * NCORES + c])
            mT = mctx[b].transpose(1, 2, 0)[:, KORDER, :Ls]        # [E, K, Ls]
            mT = np.ascontiguousarray(mT, dtype=np.float16)
            for j, korig in enumerate(KORDER):
                mT[:, j, int(lim[b, korig]):] = 0.0
            bT = np.zeros((E, Ls + PAD), np.float16)
            w = min(T, Ls + PAD)
            bT[:, :w] = bmn[b, :w].T
            in_maps[c][f"m{s}"] = mT
            in_maps[c][f"bmn{s}"] = bT

    res = bass_utils.run_bass_kernel_spmd(
        nc, in_maps, core_ids=list(range(NCORES)), trace=TRACE, **TRACE_KWARGS
    )
    LAST_RESULTS = res

    loss = 0.0
    for c in range(NCORES):
        S = res.results[c]["S"].reshape(NSLOT, K)                  # [slot, korder-idx]
        for s in range(NSLOT):
            for j, korig in enumerate(KORDER):
                loss += -S[s, j] / (B * (T - 1 - korig))
    loss /= K
    return np.float32(loss)
